# revision 61
# baseline (speedup 1.0000x reference)
"""DISCO S2 conv (DiscreteContinuousConvS2) Trainium2 Bass kernel.

Algorithm (validated vs reference in float64):
  The sparse psi tensor applied with 360 longitude shifts is a circular
  correlation along longitude.  psi is exactly even in longitude offset, so
  its longitude-DFT is purely real.  Pipeline per core:
    1. einsum over C_in fused with layout transpose:  xwT[po, m] = x[:,la,po].T @ w2
    2. forward rDFT over longitude as a matmul with a precomputed [360,362]
       cos/-sin matrix (stacked re/im), contracting po on the partition dim
    3. per-(k,dla) diagonal spectral multiply-accumulate on the Vector engine
       (10 numerically-nonzero (k,dla) pairs; 4 more are ~1e-14 and dropped;
       P-hat broadcast over output channels; fp16 scalar_tensor_tensor ops
       hit the DVE 4x perf mode)
    4. inverse rDFT as a matmul with a precomputed [362,360] matrix,
       two output channels per matmul (stationary free dim 2*how <= 128)
  All on-chip tensors are fp16 (PSUM accumulation stays fp32); fp16 matmuls
  run at 1 cycle/row on the PE regardless of output free size.
  Sharding: 8 cores = (batch b in 0..3) x (C_out half), fully data-parallel,
  no collectives.  Latitude processed in three ho-bands with +-3 la halo.
"""
import os
import sys
import numpy as np

for _p in ("/opt/trn_rl_repo",):
    if _p not in sys.path:
        sys.path.insert(0, _p)

NLAT, NLON, NF, FDIM = 181, 360, 181, 362
K, B, CIN, COUT, OH = 2, 4, 96, 96, 48
OK = OH * K
HALO = 3
# geometric band sizes: D[i] (~1.3us/row) must cover A/B[i+1] (~6+0.7/row)
# to keep the DVE fed; stage E lags one band behind D
BANDS = [(0, 38), (38, 71), (71, 117), (117, 181)]
# f-axis is repacked into three real-frequency bands (re+im interleaved per
# band) so the high-f tiles can skip polar latitudes where phat ~ 0
FB = [(0, 60), (60, 120), (120, 181)]
FS = [(0, 120), (120, 240), (240, 362)]
FPERM = np.concatenate([np.r_[fa:fb, NF + fa:NF + fb] for fa, fb in FB])
PS = [(0, 128), (128, 256), (256, 360)]
# per (pair, f-band) kept ho-range: outside it max|phat| < 3e-3*gmax.
# pair 0 stays full-range because it initializes yh.
CLIP = [
    [(0, 181), (0, 181), (0, 181)],
    [(2, 181), (9, 173), (15, 167)],
    [(0, 179), (8, 172), (14, 166)],
    [(1, 180), (12, 169), (17, 164)],
    [(2, 181), (13, 169), (21, 161)],
    [(0, 179), (12, 168), (20, 160)],
    [(3, 181), (23, 161), (36, 147)],
    [(0, 178), (20, 158), (34, 145)],
    [(4, 181), (26, 158), (43, 141)],
    [(0, 177), (23, 155), (40, 138)],
]
# (k, dla) pairs with numerically nonzero phat; (0,0) first: its phat rows
# at the poles are exactly zero, so an unclipped multiply initializes yh.
NZ = [(0, 0), (0, -1), (0, 1), (1, 0), (1, -1), (1, 1),
      (1, -2), (1, 2), (1, -3), (1, 3)]
NPAIR = len(NZ)
LAG = 5          # latitude rows per stage-A/B group (5*96 = 480 <= 512 PSUM)

_CACHE = {}
_USE_CLIP = True


def _host_prep(weight, psi_vals, k_idx, ho_idx, lat_in, lon_in):
    dla_all = lat_in.astype(np.int64) - ho_idx.astype(np.int64)
    P = np.zeros((K, 9, NLAT, NLON), dtype=np.float64)
    np.add.at(P, (k_idx, dla_all + 4, ho_idx, lon_in), psi_vals.astype(np.float64))
    f = np.arange(NF)
    ang = 2 * np.pi * np.outer(np.arange(NLON), f) / NLON          # [360,181]
    dfwd = np.concatenate([np.cos(ang), -np.sin(ang)], axis=1)     # [360,362]
    cf = np.full(NF, 2.0 / NLON)
    cf[0] = 1.0 / NLON
    cf[NF - 1] = 1.0 / NLON
    dinv = np.concatenate([cf[:, None] * np.cos(ang.T),
                           -cf[:, None] * np.sin(ang.T)], axis=0)
    dinv[NF, :] = 0.0
    dinv[2 * NF - 1, :] = 0.0                                      # [362,360]
    phat_all = P @ np.cos(ang)                                     # [K,9,NLAT,181]
    phat = np.zeros((NPAIR, FDIM, NLAT), dtype=np.float64)
    for ip, (k, dla) in enumerate(NZ):
        pT = phat_all[k, dla + 4].T                                # [181f,181ho]
        phat[ip, :NF] = pT
        phat[ip, NF:] = pT
    phatT = np.ascontiguousarray(phat.transpose(1, 0, 2))          # [362,10,181]
    # repack the f axis into real-frequency bands (see FPERM)
    dfwd = dfwd[:, FPERM]
    dinv = dinv[FPERM, :]
    phatT = phatT[FPERM]
    # guard: the hardcoded CLIP ranges must cover everything significant
    gmax = np.abs(phatT).max()
    worst = 0.0
    for ip in range(NPAIR):
        for t, (f0, f1) in enumerate(FS):
            lo, hi = CLIP[ip][t]
            m = np.abs(phatT[f0:f1, ip, :])
            if lo > 0:
                worst = max(worst, m[:, :lo].max(initial=0.0))
            if hi < NLAT:
                worst = max(worst, m[:, hi:].max(initial=0.0))
    global _USE_CLIP
    _USE_CLIP = bool(worst <= 6e-3 * gmax)
    return (np.ascontiguousarray(dfwd.astype(np.float16)),
            np.ascontiguousarray(dinv.astype(np.float16)),
            np.ascontiguousarray(phatT.astype(np.float16)))


def _build_nc():
    import concourse.bass as bass
    import concourse.bacc as bacc
    import concourse.tile as tile
    from concourse import mybir

    f16 = mybir.dt.float16
    f32 = mybir.dt.float32
    MUL = mybir.AluOpType.mult
    ADD = mybir.AluOpType.add

    nc = bacc.Bacc("TRN2", target_bir_lowering=False, debug=False)

    x_in = nc.dram_tensor("x_in", [CIN, NLAT, NLON], f16, kind="ExternalInput").ap()
    w2_in = nc.dram_tensor("w2_in", [CIN, OK], f16, kind="ExternalInput").ap()
    dfwd_in = nc.dram_tensor("dfwd_in", [NLON, FDIM], f16, kind="ExternalInput").ap()
    dinv_in = nc.dram_tensor("dinv_in", [FDIM, NLON], f16, kind="ExternalInput").ap()
    phat_in = nc.dram_tensor("phat_in", [FDIM, NPAIR, NLAT], f16,
                             kind="ExternalInput").ap()
    out_d = nc.dram_tensor("out", [OH * NLAT, NLON], f16, kind="ExternalOutput").ap()

    from contextlib import ExitStack
    with tile.TileContext(nc) as tc, ExitStack() as es:
        consts = es.enter_context(tc.tile_pool(name="consts", bufs=1))
        x_pool = es.enter_context(tc.tile_pool(name="x", bufs=3))
        xwT_pool = es.enter_context(tc.tile_pool(name="xwT", bufs=3))
        xh_pool = es.enter_context(tc.tile_pool(name="xh", bufs=2))
        yh_pool = es.enter_context(tc.tile_pool(name="yh", bufs=3))
        tmp_pool = es.enter_context(tc.tile_pool(name="tmp", bufs=3))
        tmp_pool_g = es.enter_context(tc.tile_pool(name="tmp_g", bufs=2))
        o_pool = es.enter_context(tc.tile_pool(name="o_sb", bufs=2))
        ps_a = es.enter_context(tc.tile_pool(name="ps_a", bufs=2, space=bass.MemorySpace.PSUM))
        ps_b = es.enter_context(tc.tile_pool(name="ps_b", bufs=2, space=bass.MemorySpace.PSUM))
        ps_e = es.enter_context(tc.tile_pool(name="ps_e", bufs=2, space=bass.MemorySpace.PSUM))

        w2_sb = consts.tile([CIN, OK], f16)
        nc.sync.dma_start(out=w2_sb[:, :], in_=w2_in[:, :])
        # prefetch the first two x groups ahead of the bulky constant loads
        # so band 0's stage A starts within a few us
        x_pre = []
        la0_lo = max(0, BANDS[0][0] - HALO)
        nla0 = min(NLAT, BANDS[0][1] + HALO) - la0_lo
        for g in range(0, min(2 * LAG, nla0), LAG):
            nla = min(LAG, nla0 - g)
            x_t = x_pool.tile([CIN, LAG, NLON], f16, tag="x_t")
            nc.sync.dma_start(out=x_t[:, :nla, :],
                              in_=x_in[:, la0_lo + g:la0_lo + g + nla, :])
            x_pre.append(x_t)
        dfwd_sb = consts.tile([128, 3, FDIM], f16)
        for j, (p0, p1) in enumerate(PS):
            nc.sync.dma_start(out=dfwd_sb[:p1 - p0, j, :], in_=dfwd_in[p0:p1, :])
        dinv_sb = consts.tile([128, 3, NLON], f16)
        for t, (f0, f1) in enumerate(FS):
            nc.sync.dma_start(out=dinv_sb[:f1 - f0, t, :], in_=dinv_in[f0:f1, :])
        # whole phat is only 1.3 MB in fp16 -- preload it once, so per-band
        # loads never queue behind stage-E output stores on the DMA ring
        phat_sb = consts.tile([128, 3, NPAIR, NLAT], f16)
        for t, (f0, f1) in enumerate(FS):
            nc.sync.dma_start(out=phat_sb[:f1 - f0, t, :, :],
                              in_=phat_in[f0:f1, :, :])

        def emit_E(yh, ho_lo, how):
            NQ = 8
            for quarter in range(OH // 2 // NQ):
                o_sb = o_pool.tile([128, NQ, NLON], f16, tag="o_sb")
                for q in range(NQ):
                    o = quarter * 2 * NQ + 2 * q
                    ps_o = ps_e.tile([128, NLON], f32, tag="ps_e")
                    for t, (f0, f1) in enumerate(FS):
                        fsz = f1 - f0
                        nc.tensor.matmul(
                            ps_o[:2 * how, :],
                            yh[:fsz, t, o:o + 2, :how],
                            dinv_sb[:fsz, t, :],
                            start=(t == 0), stop=(t == 2))
                    nc.scalar.copy(o_sb[:2 * how, q, :], ps_o[:2 * how, :])
                base = quarter * 2 * NQ
                for r in range(2):
                    dst = bass.AP(
                        tensor=out_d.tensor,
                        offset=((base + r) * NLAT + ho_lo) * NLON,
                        ap=[[NLON, how], [2 * NLAT * NLON, NQ], [1, NLON]])
                    nc.sync.dma_start(
                        out=dst, in_=o_sb[r * how:(r + 1) * how, :, :])

        # stage E lags TWO bands: by then D of its band is long done, so its
        # matmuls never stall the in-order PE queue ahead of the next A/B
        # KERNEL_NREP repeats the whole pipeline inside one NEFF -- used only
        # for on-device slope timing (dispatch overhead cancels exactly)
        nrep = int(os.environ.get("KERNEL_NREP", "1"))
        epending = []
        for bi, (ho_lo, ho_hi) in enumerate(BANDS * nrep):
            la_lo = max(0, ho_lo - HALO)
            la_hi = min(NLAT, ho_hi + HALO)
            nla_w = la_hi - la_lo
            how = ho_hi - ho_lo

            if len(epending) >= 2:
                emit_E(*epending.pop(0))

            xh = xh_pool.tile([128, 3, OH, K, nla_w], f16, tag="xh")

            # ---- stages A+B: einsum-T then forward DFT ----
            for g in range(0, nla_w, LAG):
                nla = min(LAG, nla_w - g)
                la0 = la_lo + g
                if x_pre:
                    x_t = x_pre.pop(0)
                else:
                    x_t = x_pool.tile([CIN, LAG, NLON], f16, tag="x_t")
                    nc.sync.dma_start(out=x_t[:, :nla, :],
                                      in_=x_in[:, la0:la0 + nla, :])
                xwT = xwT_pool.tile([128, 3, LAG, OK], f16, tag="xwT")
                for j, (p0, p1) in enumerate(PS):
                    pc = p1 - p0
                    ps_t = ps_a.tile([128, LAG, OK], f32, tag="ps_a")
                    for il in range(nla):
                        nc.tensor.matmul(
                            ps_t[:pc, il, :],
                            x_t[:, il, p0:p1],
                            w2_sb[:, :],
                            start=True, stop=True)
                    if bi == 0:
                        # DVE is idle until the first band's D starts; use it
                        # for band-0 A-copies to shorten the pipeline fill
                        nc.vector.tensor_copy(xwT[:pc, j, :nla, :],
                                              ps_t[:pc, :nla, :])
                    else:
                        nc.scalar.copy(xwT[:pc, j, :nla, :], ps_t[:pc, :nla, :])
                for t, (f0, f1) in enumerate(FS):
                    fsz = f1 - f0
                    ps_f = ps_b.tile([128, LAG, OH, K], f32, tag="ps_b")
                    for j, (p0, p1) in enumerate(PS):
                        pc = p1 - p0
                        nc.tensor.matmul(
                            ps_f[:fsz, :nla, :, :],
                            dfwd_sb[:pc, j, f0:f1],
                            xwT[:pc, j, :nla, :],
                            start=(j == 0), stop=(j == 2))
                    if (os.environ.get("KERNEL_BCOPY_DVE", "0") == "1"
                            and (g // LAG) % 2 == 0 and bi > 0):
                        nc.vector.tensor_copy(
                            xh[:fsz, t, :, :, g:g + nla],
                            ps_f[:fsz, :nla, :, :].transpose([0, 2, 3, 1]))
                    else:
                        nc.scalar.copy(
                            xh[:fsz, t, :, :, g:g + nla],
                            ps_f[:fsz, :nla, :, :].transpose([0, 2, 3, 1]))

            # ---- stage D: spectral multiply-accumulate (fp16, DVE 2x) ----
            yh = yh_pool.tile([128, 3, OH, how], f16, tag="yh")
            for t, (f0, f1) in enumerate(FS):
                fsz = f1 - f0
                for ip, (k, dla) in enumerate(NZ):
                    ho0 = max(ho_lo, -dla)
                    ho1 = min(ho_hi, NLAT - dla)
                    if _USE_CLIP and ip > 0:
                        ho0 = max(ho0, CLIP[ip][t][0])
                        ho1 = min(ho1, CLIP[ip][t][1])
                    w = ho1 - ho0
                    if w <= 0:
                        continue
                    a = ho0 + dla - la_lo
                    hl = ho0 - ho_lo
                    xs = xh[:fsz, t, :, k, a:a + w]
                    pb = phat_sb[:fsz, t, ip, ho0:ho0 + w]
                    pbc = bass.AP(
                        tensor=pb.tensor, offset=pb.offset,
                        ap=[list(pb.ap[0]), [0, OH], list(pb.ap[1])])
                    if ip == 0:
                        # (0,0): phat is exactly 0 at clipped pole rows, and
                        # dla=0 means no clipping -> full-range init multiply
                        nc.vector.tensor_mul(yh[:fsz, t, :, :], xs, pbc)
                    elif ip >= 7 and os.environ.get("KERNEL_NO_POOL", "0") != "1":
                        # offload 3 pair-multiplies to the (otherwise idle)
                        # GPSIMD engine; the chained add stays on DVE
                        tm = tmp_pool_g.tile([128, OH, how], f16, tag="tmp_g")
                        nc.gpsimd.tensor_mul(tm[:fsz, :, :w], xs, pbc)
                        nc.vector.tensor_add(
                            yh[:fsz, t, :, hl:hl + w],
                            yh[:fsz, t, :, hl:hl + w],
                            tm[:fsz, :, :w])
                    else:
                        tm = tmp_pool.tile([128, OH, how], f16, tag="tmp")
                        nc.vector.tensor_mul(tm[:fsz, :, :w], xs, pbc)
                        nc.vector.tensor_add(
                            yh[:fsz, t, :, hl:hl + w],
                            yh[:fsz, t, :, hl:hl + w],
                            tm[:fsz, :, :w])

            epending.append((yh, ho_lo, how))
        for item in epending:
            emit_E(*item)

    nc.compile()
    return nc


def _get_runner(n_cores=8):
    """Build (once) a jitted shard_map runner for the compiled Bass module.

    Mirrors concourse.bass2jax.run_bass_via_pjrt but caches the jitted
    callable so repeated kernel() calls skip retracing, and allocates the
    donated output buffers on-device instead of shipping host zeros.
    """
    if "runner" in _CACHE:
        return _CACHE["runner"]
    import jax
    import jax.numpy as jnp
    from jax.sharding import Mesh, PartitionSpec, NamedSharding
    from jax.experimental.shard_map import shard_map
    from concourse import bass2jax, mybir

    if "nc" not in _CACHE:
        _CACHE["nc"] = _build_nc()
    nc = _CACHE["nc"]
    bass2jax.install_neuronx_cc_hook()

    partition_name = (nc.partition_id_tensor.name
                      if nc.partition_id_tensor else None)
    in_names, out_names, out_avals = [], [], []
    for alloc in nc.m.functions[0].allocations:
        if not isinstance(alloc, mybir.MemoryLocationSet):
            continue
        name = alloc.memorylocations[0].name
        if alloc.kind == "ExternalInput":
            if name != partition_name:
                in_names.append(name)
        elif alloc.kind == "ExternalOutput":
            out_names.append(name)
            out_avals.append(jax.core.ShapedArray(
                tuple(alloc.tensor_shape), mybir.dt.np(alloc.dtype)))
    n_params = len(in_names)
    n_outs = len(out_avals)
    all_names = in_names + out_names
    if partition_name is not None:
        all_names = all_names + [partition_name]

    def _body(*args):
        operands = list(args)
        if partition_name is not None:
            operands.append(bass2jax.partition_id_tensor())
        outs = bass2jax._bass_exec_p.bind(
            *operands,
            out_avals=tuple(out_avals),
            in_names=tuple(all_names),
            out_names=tuple(out_names),
            lowering_input_output_aliases=(),
            sim_require_finite=True,
            sim_require_nnan=True,
            nc=nc,
        )
        return tuple(outs)

    devices = jax.devices()[:n_cores]
    mesh = Mesh(np.asarray(devices), ("core",))
    spec = PartitionSpec("core")
    sharding = NamedSharding(mesh, spec)
    donate = tuple(range(n_params, n_params + n_outs))
    sharded = jax.jit(
        shard_map(_body, mesh=mesh, in_specs=(spec,) * (n_params + n_outs),
                  out_specs=(spec,) * n_outs, check_rep=False),
        donate_argnums=donate, keep_unused=True)
    zero_shapes = [(n_cores * a.shape[0], *a.shape[1:]) for a in out_avals]
    zero_dtypes = [a.dtype for a in out_avals]
    make_zeros = jax.jit(
        lambda: tuple(jnp.zeros(s, d) for s, d in zip(zero_shapes, zero_dtypes)),
        out_shardings=(sharding,) * n_outs)
    runner = {
        "sharded": sharded, "make_zeros": make_zeros, "sharding": sharding,
        "in_names": in_names, "out_names": out_names, "out_avals": out_avals,
        "n_cores": n_cores,
    }
    _CACHE["runner"] = runner
    return runner


def _get_chain_runner(n_chain):
    """Jitted runner executing the NEFF n_chain times with output-chaining
    (each iteration consumes the previous outputs as its donated out-buffers)
    so XLA cannot CSE the repeats.  Used only for timing."""
    key = ("chain", n_chain)
    if key in _CACHE:
        return _CACHE[key]
    import jax
    from jax.sharding import Mesh, PartitionSpec
    from jax.experimental.shard_map import shard_map
    from concourse import bass2jax

    runner = _get_runner()
    nc = _CACHE["nc"]
    n_params = len(runner["in_names"])
    n_outs = len(runner["out_names"])
    out_avals = runner["out_avals"]
    partition_name = (nc.partition_id_tensor.name
                      if nc.partition_id_tensor else None)
    all_names = runner["in_names"] + runner["out_names"]
    if partition_name is not None:
        all_names = all_names + [partition_name]

    def _body(*args):
        params = list(args[:n_params])
        outs = list(args[n_params:])
        for _ in range(n_chain):
            operands = params + outs
            if partition_name is not None:
                operands.append(bass2jax.partition_id_tensor())
            outs = list(bass2jax._bass_exec_p.bind(
                *operands,
                out_avals=tuple(out_avals),
                in_names=tuple(all_names),
                out_names=tuple(runner["out_names"]),
                lowering_input_output_aliases=(),
                sim_require_finite=True,
                sim_require_nnan=True,
                nc=nc,
            ))
        return tuple(outs)

    devices = jax.devices()[:runner["n_cores"]]
    mesh = Mesh(np.asarray(devices), ("core",))
    spec = PartitionSpec("core")
    fn = jax.jit(
        shard_map(_body, mesh=mesh, in_specs=(spec,) * (n_params + n_outs),
                  out_specs=(spec,) * n_outs, check_rep=False),
        donate_argnums=tuple(range(n_params, n_params + n_outs)),
        keep_unused=True)
    _CACHE[key] = fn
    return fn


def _device_inputs(x, weight, psi_arrays):
    """Concatenated-global per-parameter arrays, device_put with sharding."""
    import jax
    dfwd, dinv, phat = _host_prep(weight, *psi_arrays)
    x16 = x.astype(np.float16)
    per_core = {"x_in": [], "w2_in": [], "dfwd_in": [], "dinv_in": [], "phat_in": []}
    for s in range(8):
        b, ohf = s // 2, s % 2
        o_sl = slice(OH * ohf, OH * ohf + OH)
        w2 = np.ascontiguousarray(
            weight[o_sl].transpose(1, 0, 2).reshape(CIN, OK)).astype(np.float16)
        per_core["x_in"].append(x16[b])
        per_core["w2_in"].append(w2)
        per_core["dfwd_in"].append(dfwd)
        per_core["dinv_in"].append(dinv)
        per_core["phat_in"].append(phat)
    runner = _get_runner()
    concat = {k: np.concatenate(v, axis=0) for k, v in per_core.items()}
    return [jax.device_put(concat[name], runner["sharding"])
            for name in runner["in_names"]]


def _run_device(dev_in):
    runner = _get_runner()
    zeros = runner["make_zeros"]()
    return runner["sharded"](*dev_in, *zeros)


def kernel(x, weight, bias, psi_vals, k_idx, ho_idx, lat_in_idx, lon_in_idx):
    x = np.ascontiguousarray(np.asarray(x, dtype=np.float32))
    weight = np.asarray(weight, dtype=np.float32)
    bias = np.asarray(bias, dtype=np.float32)
    psi_arrays = (np.asarray(psi_vals), np.asarray(k_idx), np.asarray(ho_idx),
                  np.asarray(lat_in_idx), np.asarray(lon_in_idx))

    dev_in = _device_inputs(x, weight, psi_arrays)
    out_arrs = _run_device(dev_in)
    runner = _get_runner()
    a0 = runner["out_avals"][0]
    res0 = np.asarray(out_arrs[0]).reshape(8, *a0.shape)

    out = np.empty((B, COUT, NLAT, NLON), dtype=np.float32)
    for s in range(8):
        b, ohf = s // 2, s % 2
        out[b, OH * ohf:OH * ohf + OH] = res0[s].reshape(OH, NLAT, NLON)
    if np.any(bias):
        out += bias[None, :, None, None]
    return out
